# revision 69
# baseline (speedup 1.0000x reference)
"""ContextualLoss forward on 8 Trainium2 NeuronCores.

Math (reference):
    mu[m]   = mean_c Y[c, m]                      (PONO over channels of Y)
    Xc = X - mu ; Yc = Y - mu                     (both centered by Y's mean)
    cos[i,j] = <Xc_i, Yc_j> / (|Xc_i| |Yc_j|)
    d = 1 - cos ; dn = d / (min_j d + 1e-3) ; w = exp((1 - dn)/0.1)
    A = w / sum_j w ; CX_b = mean_i max_j A ; loss = mean_b -log CX_b

Device-side tricks:
  * Only Y is centered explicitly. Since Yc has zero channel-mean,
    <Xc_i, Yc_j> == <X_i, Yc_j>, so raw X feeds the matmul.
  * max_j A = exp-at-dmin / sum_j w = exp(0.01/(dmin+1e-3)) / sum_j w
    (w is monotone decreasing in d) -> no second max pass over w.
  * The per-column scale 1/|Yc_j| is pre-applied to the centered Y tile
    (bf16, 2x DVE mode), so PSUM holds d = cos*|Xc_i| directly. The
    PSUM->SBUF drain is split: ScalarE Identity for quarters 0/1, DVE
    copy for quarters 2/3; row maxes via DVE reduce_max on the bf16 d.
    (The fused tensor_tensor_reduce path crashes at runtime on this
    toolchain, so the drain is unfused by design.)
  * Per-row scale 1/|Xc_i| and the softmin exponent fold into the ScalarE
    activation: w = Exp(scale_i * dsc + bias_i), scale_i = s*inv_nx,
    bias_i = 10 - s, s = 10/(dmin+1e-3); accum_out gives sum_j w for free.

Sharding: core c -> sample b = c//2, row-half h = c%2 (2048 rows each).
Each core's Y is column-permuted host-side to [own-half | other-half] so the
identical SPMD program can read the X-half's means from columns [0, 2048).
Row reductions are permutation-invariant, so the permutation is harmless.
"""

import os
import sys
from contextlib import ExitStack

sys.path.insert(0, "/opt/trn_rl_repo")

import numpy as np

import concourse.bass as bass
import concourse.tile as tile
from concourse import bacc
from concourse import mybir
from concourse.bass_utils import run_bass_kernel_spmd

B = 4
C = 256
M = 4096  # 64*64 spatial positions
HALF = M // 2  # rows per core
NT = HALF // 128  # 16 i-tiles per core
N_CORES = 8

F32 = mybir.dt.float32
F32R = mybir.dt.float32r
BF16 = mybir.dt.bfloat16
AF = mybir.ActivationFunctionType
ALU = mybir.AluOpType

NEG_HUGE = -3.0e38


def _r(ap):
    """View a fp32 AP as float32r for full-rate PE matmul."""
    return ap.bitcast(F32R)


def build_nc() -> bass.Bass:
    nc = bacc.Bacc()

    x_d = nc.declare_dram_parameter("x", [C, HALF], F32, isOutput=False)
    y_d = nc.declare_dram_parameter("y", [C, M], F32, isOutput=False)
    v_d = nc.declare_dram_parameter("v", [128, NT], F32, isOutput=True)

    Q = 1024  # preprocessing quarter width

    with tile.TileContext(nc) as tc:
        with (
            tc.tile_pool(name="io", bufs=1) as io,
            tc.tile_pool(name="consts", bufs=1) as consts,
            tc.tile_pool(name="stats", bufs=1) as stats,
        ):
            # ---- inputs -> SBUF: y half 0 first (feeds the sy/center
            # chain), then x, then y half 1 ------------------------------
            x_bf = io.tile([128, 2, HALF], BF16)
            y_bf = io.tile([128, 2, M], BF16)

            # ---- constants ------------------------------------------------
            ones_col = consts.tile([128, 1], F32)
            nc.vector.memset(ones_col, 1.0)
            ones_col_bf = consts.tile([128, 1], BF16)
            nc.vector.memset(ones_col_bf, 1.0)
            ones_row = consts.tile([1, 128], F32)
            nc.vector.memset(ones_row, 1.0)
            ones_row_bf = consts.tile([1, 128], BF16)
            nc.vector.memset(ones_row_bf, 1.0)
            inv256_row_bf = consts.tile([1, 128], BF16)
            nc.vector.memset(inv256_row_bf, 1.0 / 256.0)
            oc256_bf = consts.tile([128, 128], BF16)
            nc.vector.memset(oc256_bf, 1.0 / 256.0)
            ten_col = consts.tile([128, 1], F32)
            nc.vector.memset(ten_col, 10.0)
            one_1x1 = consts.tile([1, 1], F32)
            nc.vector.memset(one_1x1, 1.0)
            one_1x1_bf = consts.tile([1, 1], BF16)
            nc.vector.memset(one_1x1_bf, 1.0)

            inv_ny_b = io.tile([128, M], F32)  # |Yc| then 1/|Yc| broadcast
            inv_ny_bf = io.tile([128, M], BF16)  # bf16 copy for pre-scaling

            nx2 = stats.tile([128, NT], F32)
            inv_nx = stats.tile([128, NT], F32)
            inv_nx10 = stats.tile([128, NT], F32)
            r16 = stats.tile([128, NT], F32)
            sumw16 = stats.tile([128, NT], F32)
            maxw16 = stats.tile([128, NT], F32)
            rs16 = stats.tile([128, NT], F32)
            v16 = stats.tile([128, NT], F32)
            t_b = stats.tile([128, NT], F32)

            # main-loop pools opened alongside preprocessing so tile-0
            # quarters can interleave with the tail of preprocessing
            mstack = ExitStack()
            dpool = mstack.enter_context(tc.tile_pool(name="dpool", bufs=6))
            wpool = mstack.enter_context(tc.tile_pool(name="wpool", bufs=1))
            mains = mstack.enter_context(tc.tile_pool(name="mains", bufs=4))
            psum_g = mstack.enter_context(tc.tile_pool(name="psum_g", bufs=2, space="PSUM"))

            NEARLY = 4
            d_sbs, cmaxs = {}, {}
            for _t in range(NEARLY):
                d_sbs[_t] = dpool.tile([128, M], BF16, tag="d_sb", name=f"d_sb{_t}")
                cmaxs[_t] = (
                    mains.tile([128, Q], BF16, tag="tmp01", name=f"tmp01_{_t}"),
                    mains.tile([128, Q], BF16, tag="tmp23", name=f"tmp23_{_t}"),
                )

            def g_quarter(t, g, d_tile, cmax_tile, gpool=None):
                # y_bf is pre-scaled by 1/|Yc|, so PSUM holds d = cos*|Xc_i|
                # directly: quarters 0/1 drain on Act (Identity), 2/3 on DVE.
                ps = (gpool or psum_g).tile([128, Q], F32, tag="g")
                for k in range(2):
                    for j in range(2):
                        nc.tensor.matmul(
                            ps[:, j * 512 : (j + 1) * 512],
                            lhsT=x_bf[:, k, t * 128 : (t + 1) * 128],
                            rhs=y_bf[:, k, g * Q + j * 512 : g * Q + (j + 1) * 512],
                            start=(k == 0),
                            stop=(k == 1),
                        )
                if g == 0 or (g == 1 and t % 4 != 3):
                    nc.scalar.activation(
                        d_tile[:, g * Q : (g + 1) * Q], ps[:, :], AF.Identity
                    )
                else:
                    nc.vector.tensor_copy(
                        d_tile[:, g * Q : (g + 1) * Q], ps[:, :]
                    )
                # pairwise max tree: bf16 all-SBUF tensor_max runs at 2x
                if g == 1:
                    nc.vector.tensor_max(
                        cmax_tile[0][:, :], d_tile[:, 0:Q], d_tile[:, Q : 2 * Q]
                    )
                elif g == 3:
                    nc.vector.tensor_max(
                        cmax_tile[1][:, :], d_tile[:, 2 * Q : 3 * Q], d_tile[:, 3 * Q : 4 * Q]
                    )

            def tile_chain_exp(t, d_sb, cmax2):
                cmax = mains.tile([128, 1], F32)
                u = mains.tile([128, 1], F32)
                bias_i = mains.tile([128, 1], F32)
                scale_i = mains.tile([128, 1], F32)
                nc.vector.tensor_max(cmax2[0][:, :], cmax2[0][:, :], cmax2[1][:, :])
                nc.vector.reduce_max(cmax[:, :], cmax2[0][:, :], axis=mybir.AxisListType.X)
                nc.vector.tensor_mul(cmax[:, :], cmax[:, :], inv_nx[:, t : t + 1])
                nc.vector.tensor_scalar(
                    out=u[:, :],
                    in0=cmax[:, :],
                    scalar1=-1.0,
                    scalar2=1.001,
                    op0=ALU.mult,
                    op1=ALU.add,
                )
                nc.vector.reciprocal(r16[:, t : t + 1], u[:, :])
                nc.vector.tensor_mul(
                    scale_i[:, :], r16[:, t : t + 1], inv_nx10[:, t : t + 1]
                )
                nc.vector.tensor_scalar(
                    out=bias_i[:, :], in0=r16[:, t : t + 1],
                    scalar1=-10.0, scalar2=10.0,
                    op0=ALU.mult, op1=ALU.add,
                )
                w_sb = wpool.tile([128, M], BF16)
                nc.scalar.activation(
                    out=w_sb[:, :],
                    in_=d_sb[:, :],
                    func=AF.Exp,
                    bias=bias_i[:, :],
                    scale=scale_i[:, :],
                    accum_out=sumw16[:, t : t + 1],
                )


            with (
                tc.tile_pool(name="psum_pre", bufs=2, space="PSUM") as pre,
                tc.tile_pool(name="rows", bufs=1) as rows,
                tc.tile_pool(name="scratch", bufs=3) as scratch,
            ):
                # PE warmup to full pstate while DMAs are in flight
                wups = pre.tile([128, 512], F32, tag="pre")
                dumw = consts.tile([128, 1], F32)
                for _wi in range(50):
                    nc.tensor.matmul(
                        wups[:, (_wi % 4) * 128 : (_wi % 4 + 1) * 128],
                        lhsT=ones_row_bf[0:1, :],
                        rhs=ones_row_bf[0:1, :],
                        start=True,
                        stop=True,
                    )
                # Act table preload: Sqrt-set covers Square/Identity/Copy
                nc.scalar.activation(dumw[:, :], ten_col[:, :], AF.Sqrt)
                sy_row = rows.tile([1, M], BF16, tag="rowM")
                qy_row = rows.tile([1, M], BF16, tag="rowM2")
                sq = rows.tile([128, 2, M], BF16)  # squares staging (bf16: only feeds aggregate norms)

                ystage = {}

                y_v = y_d.rearrange("(k p) m -> p k m", p=128)
                x_v = x_d.rearrange("(k p) m -> p k m", p=128)

                def y_dma_quarter(q):
                    st = scratch.tile([128, 2, Q], F32, tag="stage")
                    nc.sync.dma_start(
                        out=st[:, :, :], in_=y_v[:, :, q * Q : (q + 1) * Q]
                    )
                    # raw y -> bf16 (centered in place later)
                    nc.vector.tensor_copy(
                        y_bf[:, :, q * Q : (q + 1) * Q], st[:, :, :]
                    )
                    ystage[q] = st

                def x_quarter(q):
                    st = scratch.tile([128, 2, Q], F32, tag="stage")
                    nc.sync.dma_start(
                        out=st[:, :, :], in_=x_v[:, :, q * Q : (q + 1) * Q]
                    )
                    # Pool is idle in the preamble; keep Act free for the
                    # y-quarter chains
                    nc.gpsimd.tensor_copy(
                        x_bf[:, :, q * Q : (q + 1) * Q], st[:, :, :]
                    )

                def xcenter_quarter(q):
                    # x_bf -= mu (in place); then xc^2 -> sq (overwrites y^2
                    # region after qy MMs have consumed it)
                    ps = pre.tile([128, Q], F32, tag="pre")
                    for j in range(2):
                        nc.tensor.matmul(
                            ps[:, j * 512 : (j + 1) * 512],
                            lhsT=inv256_row_bf[:, :],
                            rhs=sy_row[:, q * Q + j * 512 : q * Q + (j + 1) * 512],
                            start=True,
                            stop=True,
                        )
                    for k in range(2):
                        nc.vector.tensor_sub(
                            x_bf[:, k, q * Q : (q + 1) * Q],
                            x_bf[:, k, q * Q : (q + 1) * Q],
                            ps[:, :],
                        )
                    nc.scalar.activation(
                        sq[:, :, q * Q : (q + 1) * Q],
                        x_bf[:, :, q * Q : (q + 1) * Q],
                        AF.Square,
                    )

                def sy_quarter(q):
                    ps = pre.tile([1, Q], F32, tag="pre")
                    for k in range(2):
                        for j in range(2):
                            nc.tensor.matmul(
                                ps[:, j * 512 : (j + 1) * 512],
                                lhsT=ones_col_bf[:, :],
                                rhs=y_bf[:, k, q * Q + j * 512 : q * Q + (j + 1) * 512],
                                start=(k == 0),
                                stop=(k == 1),
                            )
                    nc.scalar.copy(sy_row[:, q * Q : (q + 1) * Q], ps[:, :])

                def center_quarter(q):
                    # mu broadcast (1/256 via lhsT), subtract into bf16, then y^2
                    ps = pre.tile([128, Q], F32, tag="pre")
                    for j in range(2):
                        nc.tensor.matmul(
                            ps[:, j * 512 : (j + 1) * 512],
                            lhsT=inv256_row_bf[:, :],
                            rhs=sy_row[:, q * Q + j * 512 : q * Q + (j + 1) * 512],
                            start=True,
                            stop=True,
                        )
                    for k in range(2):
                        nc.vector.tensor_sub(
                            y_bf[:, k, q * Q : (q + 1) * Q],
                            y_bf[:, k, q * Q : (q + 1) * Q],
                            ps[:, :],
                        )
                    nc.scalar.activation(
                        sq[:, :, q * Q : (q + 1) * Q],
                        y_bf[:, :, q * Q : (q + 1) * Q],
                        AF.Square,
                    )

                def center_direct(q):
                    # mu broadcast straight from raw y via ones/256 matmul
                    # (skips the sy row + its Act copy for quarters 2/3)
                    ps = pre.tile([128, Q], F32, tag="pre")
                    for j in range(2):
                        for k in range(2):
                            nc.tensor.matmul(
                                ps[:, j * 512 : (j + 1) * 512],
                                lhsT=oc256_bf[:, :],
                                rhs=y_bf[:, k, q * Q + j * 512 : q * Q + (j + 1) * 512],
                                start=(k == 0),
                                stop=(k == 1),
                            )
                    for k in range(2):
                        nc.vector.tensor_sub(
                            y_bf[:, k, q * Q : (q + 1) * Q],
                            y_bf[:, k, q * Q : (q + 1) * Q],
                            ps[:, :],
                        )
                    nc.scalar.activation(
                        sq[:, :, q * Q : (q + 1) * Q],
                        y_bf[:, :, q * Q : (q + 1) * Q],
                        AF.Square,
                    )

                def qy_quarter(q):
                    ps = pre.tile([1, Q], F32, tag="pre")
                    for k in range(2):
                        for j in range(2):
                            nc.tensor.matmul(
                                ps[:, j * 512 : (j + 1) * 512],
                                lhsT=ones_col_bf[:, :],
                                rhs=sq[:, k, q * Q + j * 512 : q * Q + (j + 1) * 512],
                                start=(k == 0),
                                stop=(k == 1),
                            )
                    nc.scalar.copy(qy_row[:, q * Q : (q + 1) * Q], ps[:, :])

                def invb_quarter(q):
                    ps = pre.tile([128, Q], F32, tag="pre")
                    for j in range(2):
                        nc.tensor.matmul(
                            ps[:, j * 512 : (j + 1) * 512],
                            lhsT=ones_row_bf[:, :],
                            rhs=qy_row[:, q * Q + j * 512 : q * Q + (j + 1) * 512],
                            start=True,
                            stop=True,
                        )
                    nc.scalar.activation(
                        inv_ny_b[:, q * Q : (q + 1) * Q], ps[:, :], AF.Sqrt
                    )
                    with nc.allow_low_precision(reason="inv_ny to bf16 for prescale"):
                        nc.vector.reciprocal(
                            inv_ny_bf[:, q * Q : (q + 1) * Q],
                            inv_ny_b[:, q * Q : (q + 1) * Q],
                        )
                    # pre-scale centered y columns by 1/|Yc| (bf16 2x mode)
                    for k in range(2):
                        nc.vector.tensor_mul(
                            y_bf[:, k, q * Q : (q + 1) * Q],
                            y_bf[:, k, q * Q : (q + 1) * Q],
                            inv_ny_bf[:, q * Q : (q + 1) * Q],
                        )


                def stat16(dst16, src_tile):
                    # dst16[p, t] = sum_c src[c, t*128+p] via N=1 matmuls
                    ps = pre.tile([128, NT], F32, tag="pre")
                    for t in range(NT):
                        for k in range(2):
                            nc.tensor.matmul(
                                ps[:, t : t + 1],
                                lhsT=src_tile[:, k, t * 128 : (t + 1) * 128],
                                rhs=ones_col_bf[:, :],
                                start=(k == 0),
                                stop=(k == 1),
                            )
                    nc.vector.tensor_copy(dst16[:, :], ps[:, :])

                # ---- phase schedule (program order ~ priority) ----------
                y_dma_quarter(0)
                y_dma_quarter(1)
                sy_quarter(0)
                sy_quarter(1)
                center_quarter(0)
                center_quarter(1)
                qy_quarter(0)
                invb_quarter(0)
                qy_quarter(1)
                invb_quarter(1)
                x_quarter(0)
                x_quarter(1)
                xcenter_quarter(0)
                xcenter_quarter(1)
                if os.environ.get("BISECT", "") != "pre":
                    g_quarter(0, 0, d_sbs[0], cmaxs[0])
                stat16(nx2, sq)
                # inv_nx from nx2 (already tile-major)
                nc.scalar.activation(t_b[:, :], nx2[:, :], AF.Sqrt)
                nc.vector.reciprocal(inv_nx[:, :], t_b[:, :])
                nc.vector.tensor_scalar_mul(inv_nx10[:, :], inv_nx[:, :], 10.0)
                y_dma_quarter(2)
                y_dma_quarter(3)
                if os.environ.get("BISECT", "") != "pre":
                    g_quarter(0, 1, d_sbs[0], cmaxs[0])
                center_direct(2)
                center_direct(3)
                qy_quarter(2)
                invb_quarter(2)
                qy_quarter(3)
                invb_quarter(3)
                if os.environ.get("BISECT", "") != "pre":
                    g_quarter(0, 2, d_sbs[0], cmaxs[0])
                    g_quarter(0, 3, d_sbs[0], cmaxs[0])
                    tile_chain_exp(0, d_sbs[0], cmaxs[0])


            # ---- main loop (pools opened above; t=0 quarters already
            # issued inside preprocessing) --------------------------------

            # the preprocessing PSUM pool is closed: its banks back a third
            # and fourth in-flight G buffer for the steady-state loop
            with tc.tile_pool(name="psum_g2", bufs=2, space="PSUM") as psum_g2:
                for t in (range(1, NT) if os.environ.get("BISECT", "") != "pre" else range(0)):
                    d_sb = dpool.tile([128, M], BF16, tag="d_sb")
                    tmp01 = mains.tile([128, Q], BF16, tag="tmp01")
                    tmp23 = mains.tile([128, Q], BF16, tag="tmp23")
                    cmax2 = (tmp01, tmp23)
                    for g in range(4):
                        g_quarter(t, g, d_sb, cmax2,
                                  gpool=psum_g2 if g >= 2 else None)
                    tile_chain_exp(t, d_sb, cmax2)

            # ---- epilogue: v = exp(0.01*r) / sumw -----------------------
            if os.environ.get("BISECT", "") == "pre":
                nc.vector.tensor_copy(v16[:, :], inv_nx[:, :])
            else:
                nc.scalar.activation(maxw16[:, :], r16[:, :], AF.Exp, scale=0.01)
                nc.vector.reciprocal(rs16[:, :], sumw16[:, :])
                nc.vector.tensor_mul(v16[:, :], maxw16[:, :], rs16[:, :])
            nc.sync.dma_start(out=v_d[:, :], in_=v16[:, :])

            mstack.close()

    nc.compile()
    return nc

_NC = None


def _get_nc():
    global _NC
    if _NC is None:
        _NC = build_nc()
    return _NC


def make_in_maps(X, Y):
    """Per-core inputs. Y columns permuted to [own-half | other-half]."""
    in_maps = []
    for c in range(N_CORES):
        b, h = c // 2, c % 2
        xs = np.ascontiguousarray(X[b][:, h * HALF : (h + 1) * HALF])
        ys = np.ascontiguousarray(
            np.concatenate(
                [
                    Y[b][:, h * HALF : (h + 1) * HALF],
                    Y[b][:, (1 - h) * HALF : (2 - h) * HALF],
                ],
                axis=1,
            )
        )
        in_maps.append({"x": xs, "y": ys})
    return in_maps


def finish_host(results):
    """results: list of 8 per-core dicts with 'v' [128, NT]."""
    cx = np.zeros(B, dtype=np.float64)
    for c in range(N_CORES):
        cx[c // 2] += results[c]["v"].astype(np.float64).sum()
    cx /= M
    return np.float32(np.mean(-np.log(cx)))


def run(X_features, Y_features, trace=False, tmpdir=None):
    X = np.asarray(X_features, dtype=np.float32).reshape(B, C, M)
    Y = np.asarray(Y_features, dtype=np.float32).reshape(B, C, M)
    nc = _get_nc()
    res = run_bass_kernel_spmd(
        nc, make_in_maps(X, Y), list(range(N_CORES)), trace=trace, tmpdir=tmpdir
    )
    return finish_host(res.results), res


def kernel(X_features, Y_features):
    loss, _ = run(X_features, Y_features)
    return loss



# revision 74
# speedup vs baseline: 1.0016x; 1.0016x over previous
"""ContextualLoss forward on 8 Trainium2 NeuronCores.

Math (reference):
    mu[m]   = mean_c Y[c, m]                      (PONO over channels of Y)
    Xc = X - mu ; Yc = Y - mu                     (both centered by Y's mean)
    cos[i,j] = <Xc_i, Yc_j> / (|Xc_i| |Yc_j|)
    d = 1 - cos ; dn = d / (min_j d + 1e-3) ; w = exp((1 - dn)/0.1)
    A = w / sum_j w ; CX_b = mean_i max_j A ; loss = mean_b -log CX_b

Device-side tricks:
  * Only Y is centered explicitly. Since Yc has zero channel-mean,
    <Xc_i, Yc_j> == <X_i, Yc_j>, so raw X feeds the matmul.
  * max_j A = exp-at-dmin / sum_j w = exp(0.01/(dmin+1e-3)) / sum_j w
    (w is monotone decreasing in d) -> no second max pass over w.
  * The per-column scale 1/|Yc_j| is pre-applied to the centered Y tile
    (bf16, 2x DVE mode), so PSUM holds d = cos*|Xc_i| directly. The
    PSUM->SBUF drain is split: ScalarE Identity for quarters 0/1, DVE
    copy for quarters 2/3; row maxes via DVE reduce_max on the bf16 d.
    (The fused tensor_tensor_reduce path crashes at runtime on this
    toolchain, so the drain is unfused by design.)
  * Per-row scale 1/|Xc_i| and the softmin exponent fold into the ScalarE
    activation: w = Exp(scale_i * dsc + bias_i), scale_i = s*inv_nx,
    bias_i = 10 - s, s = 10/(dmin+1e-3); accum_out gives sum_j w for free.

Sharding: core c -> sample b = c//2, row-half h = c%2 (2048 rows each).
Each core's Y is column-permuted host-side to [own-half | other-half] so the
identical SPMD program can read the X-half's means from columns [0, 2048).
Row reductions are permutation-invariant, so the permutation is harmless.
"""

import os
import sys
from contextlib import ExitStack

sys.path.insert(0, "/opt/trn_rl_repo")

import numpy as np

import concourse.bass as bass
import concourse.tile as tile
from concourse import bacc
from concourse import mybir
from concourse.bass_utils import run_bass_kernel_spmd

B = 4
C = 256
M = 4096  # 64*64 spatial positions
HALF = M // 2  # rows per core
NT = HALF // 128  # 16 i-tiles per core
N_CORES = 8

F32 = mybir.dt.float32
F32R = mybir.dt.float32r
BF16 = mybir.dt.bfloat16
AF = mybir.ActivationFunctionType
ALU = mybir.AluOpType

NEG_HUGE = -3.0e38


def _r(ap):
    """View a fp32 AP as float32r for full-rate PE matmul."""
    return ap.bitcast(F32R)


def build_nc() -> bass.Bass:
    nc = bacc.Bacc()

    x_d = nc.declare_dram_parameter("x", [C, HALF], F32, isOutput=False)
    y_d = nc.declare_dram_parameter("y", [C, M], F32, isOutput=False)
    v_d = nc.declare_dram_parameter("v", [128, NT], F32, isOutput=True)

    Q = 1024  # preprocessing quarter width

    with tile.TileContext(nc) as tc:
        with (
            tc.tile_pool(name="io", bufs=1) as io,
            tc.tile_pool(name="consts", bufs=1) as consts,
            tc.tile_pool(name="stats", bufs=1) as stats,
        ):
            # ---- inputs -> SBUF: y half 0 first (feeds the sy/center
            # chain), then x, then y half 1 ------------------------------
            x_bf = io.tile([128, 2, HALF], BF16)
            y_bf = io.tile([128, 2, M], BF16)

            # ---- constants ------------------------------------------------
            ones_col = consts.tile([128, 1], F32)
            nc.vector.memset(ones_col, 1.0)
            ones_col_bf = consts.tile([128, 1], BF16)
            nc.vector.memset(ones_col_bf, 1.0)
            ones_row = consts.tile([1, 128], F32)
            nc.vector.memset(ones_row, 1.0)
            ones_row_bf = consts.tile([1, 128], BF16)
            nc.vector.memset(ones_row_bf, 1.0)
            inv256_row_bf = consts.tile([1, 128], BF16)
            nc.vector.memset(inv256_row_bf, 1.0 / 256.0)
            oc256_bf = consts.tile([128, 128], BF16)
            nc.vector.memset(oc256_bf, 1.0 / 256.0)
            ten_col = consts.tile([128, 1], F32)
            nc.vector.memset(ten_col, 10.0)
            one_1x1 = consts.tile([1, 1], F32)
            nc.vector.memset(one_1x1, 1.0)
            one_1x1_bf = consts.tile([1, 1], BF16)
            nc.vector.memset(one_1x1_bf, 1.0)

            inv_ny_b = io.tile([128, M], F32)  # |Yc| then 1/|Yc| broadcast
            inv_ny_bf = io.tile([128, M], BF16)  # bf16 copy for pre-scaling

            nx2 = stats.tile([128, NT], F32)
            inv_nx = stats.tile([128, NT], F32)
            inv_nx10 = stats.tile([128, NT], F32)
            r16 = stats.tile([128, NT], F32)
            sumw16 = stats.tile([128, NT], F32)
            maxw16 = stats.tile([128, NT], F32)
            rs16 = stats.tile([128, NT], F32)
            v16 = stats.tile([128, NT], F32)
            t_b = stats.tile([128, NT], F32)

            # main-loop pools opened alongside preprocessing so tile-0
            # quarters can interleave with the tail of preprocessing
            mstack = ExitStack()
            dpool = mstack.enter_context(tc.tile_pool(name="dpool", bufs=6))
            wpool = mstack.enter_context(tc.tile_pool(name="wpool", bufs=1))
            mains = mstack.enter_context(tc.tile_pool(name="mains", bufs=4))
            psum_g = mstack.enter_context(tc.tile_pool(name="psum_g", bufs=2, space="PSUM"))

            NEARLY = 4
            d_sbs, cmaxs = {}, {}
            for _t in range(NEARLY):
                d_sbs[_t] = dpool.tile([128, M], BF16, tag="d_sb", name=f"d_sb{_t}")
                cmaxs[_t] = (
                    mains.tile([128, Q], BF16, tag="tmp01", name=f"tmp01_{_t}"),
                    mains.tile([128, Q], BF16, tag="tmp23", name=f"tmp23_{_t}"),
                )

            def g_quarter(t, g, d_tile, cmax_tile, gpool=None):
                # y_bf is pre-scaled by 1/|Yc|, so PSUM holds d = cos*|Xc_i|
                # directly: quarters 0/1 drain on Act (Identity), 2/3 on DVE.
                ps = (gpool or psum_g).tile([128, Q], F32, tag="g")
                for k in range(2):
                    for j in range(2):
                        nc.tensor.matmul(
                            ps[:, j * 512 : (j + 1) * 512],
                            lhsT=x_bf[:, k, t * 128 : (t + 1) * 128],
                            rhs=y_bf[:, k, g * Q + j * 512 : g * Q + (j + 1) * 512],
                            start=(k == 0),
                            stop=(k == 1),
                        )
                if g == 0 or (g == 1 and t % 4 != 3):
                    nc.scalar.activation(
                        d_tile[:, g * Q : (g + 1) * Q], ps[:, :], AF.Identity
                    )
                else:
                    nc.vector.tensor_copy(
                        d_tile[:, g * Q : (g + 1) * Q], ps[:, :]
                    )
                # pairwise max tree: bf16 all-SBUF tensor_max runs at 2x
                if g == 1:
                    nc.vector.tensor_max(
                        cmax_tile[0][:, :], d_tile[:, 0:Q], d_tile[:, Q : 2 * Q]
                    )
                elif g == 3:
                    nc.vector.tensor_max(
                        cmax_tile[1][:, :], d_tile[:, 2 * Q : 3 * Q], d_tile[:, 3 * Q : 4 * Q]
                    )

            def tile_chain_exp(t, d_sb, cmax2):
                cmax = mains.tile([128, 1], F32)
                u = mains.tile([128, 1], F32)
                bias_i = mains.tile([128, 1], F32)
                scale_i = mains.tile([128, 1], F32)
                nc.vector.tensor_max(cmax2[0][:, :], cmax2[0][:, :], cmax2[1][:, :])
                nc.vector.tensor_max(
                    cmax2[0][:, 0:512], cmax2[0][:, 0:512], cmax2[0][:, 512:1024]
                )
                nc.vector.tensor_max(
                    cmax2[0][:, 0:256], cmax2[0][:, 0:256], cmax2[0][:, 256:512]
                )
                nc.vector.reduce_max(cmax[:, :], cmax2[0][:, 0:256], axis=mybir.AxisListType.X)
                nc.vector.tensor_mul(cmax[:, :], cmax[:, :], inv_nx[:, t : t + 1])
                nc.vector.tensor_scalar(
                    out=u[:, :],
                    in0=cmax[:, :],
                    scalar1=-1.0,
                    scalar2=1.001,
                    op0=ALU.mult,
                    op1=ALU.add,
                )
                nc.vector.reciprocal(r16[:, t : t + 1], u[:, :])
                nc.vector.tensor_mul(
                    scale_i[:, :], r16[:, t : t + 1], inv_nx10[:, t : t + 1]
                )
                nc.vector.tensor_scalar(
                    out=bias_i[:, :], in0=r16[:, t : t + 1],
                    scalar1=-10.0, scalar2=10.0,
                    op0=ALU.mult, op1=ALU.add,
                )
                w_sb = wpool.tile([128, M], BF16)
                nc.scalar.activation(
                    out=w_sb[:, :],
                    in_=d_sb[:, :],
                    func=AF.Exp,
                    bias=bias_i[:, :],
                    scale=scale_i[:, :],
                    accum_out=sumw16[:, t : t + 1],
                )


            with (
                tc.tile_pool(name="psum_pre", bufs=2, space="PSUM") as pre,
                tc.tile_pool(name="rows", bufs=1) as rows,
                tc.tile_pool(name="scratch", bufs=3) as scratch,
            ):
                # PE warmup to full pstate while DMAs are in flight
                wups = pre.tile([128, 512], F32, tag="pre")
                dumw = consts.tile([128, 1], F32)
                for _wi in range(50):
                    nc.tensor.matmul(
                        wups[:, (_wi % 4) * 128 : (_wi % 4 + 1) * 128],
                        lhsT=ones_row_bf[0:1, :],
                        rhs=ones_row_bf[0:1, :],
                        start=True,
                        stop=True,
                    )
                # Act table preload: Sqrt-set covers Square/Identity/Copy
                nc.scalar.activation(dumw[:, :], ten_col[:, :], AF.Sqrt)
                sy_row = rows.tile([1, M], BF16, tag="rowM")
                qy_row = rows.tile([1, M], BF16, tag="rowM2")
                sq = rows.tile([128, 2, M], BF16)  # squares staging (bf16: only feeds aggregate norms)

                ystage = {}

                y_v = y_d.rearrange("(k p) m -> p k m", p=128)
                x_v = x_d.rearrange("(k p) m -> p k m", p=128)

                def y_dma_quarter(q):
                    st = scratch.tile([128, 2, Q], F32, tag="stage")
                    nc.sync.dma_start(
                        out=st[:, :, :], in_=y_v[:, :, q * Q : (q + 1) * Q]
                    )
                    # raw y -> bf16 (centered in place later)
                    nc.vector.tensor_copy(
                        y_bf[:, :, q * Q : (q + 1) * Q], st[:, :, :]
                    )
                    ystage[q] = st

                def x_quarter(q):
                    st = scratch.tile([128, 2, Q], F32, tag="stage")
                    nc.sync.dma_start(
                        out=st[:, :, :], in_=x_v[:, :, q * Q : (q + 1) * Q]
                    )
                    # Pool is idle in the preamble; keep Act free for the
                    # y-quarter chains
                    nc.gpsimd.tensor_copy(
                        x_bf[:, :, q * Q : (q + 1) * Q], st[:, :, :]
                    )

                def xcenter_quarter(q):
                    # x_bf -= mu (in place); then xc^2 -> sq (overwrites y^2
                    # region after qy MMs have consumed it)
                    ps = pre.tile([128, Q], F32, tag="pre")
                    for j in range(2):
                        nc.tensor.matmul(
                            ps[:, j * 512 : (j + 1) * 512],
                            lhsT=inv256_row_bf[:, :],
                            rhs=sy_row[:, q * Q + j * 512 : q * Q + (j + 1) * 512],
                            start=True,
                            stop=True,
                        )
                    for k in range(2):
                        nc.vector.tensor_sub(
                            x_bf[:, k, q * Q : (q + 1) * Q],
                            x_bf[:, k, q * Q : (q + 1) * Q],
                            ps[:, :],
                        )
                    nc.scalar.activation(
                        sq[:, :, q * Q : (q + 1) * Q],
                        x_bf[:, :, q * Q : (q + 1) * Q],
                        AF.Square,
                    )

                def sy_quarter(q):
                    ps = pre.tile([1, Q], F32, tag="pre")
                    for k in range(2):
                        for j in range(2):
                            nc.tensor.matmul(
                                ps[:, j * 512 : (j + 1) * 512],
                                lhsT=ones_col_bf[:, :],
                                rhs=y_bf[:, k, q * Q + j * 512 : q * Q + (j + 1) * 512],
                                start=(k == 0),
                                stop=(k == 1),
                            )
                    nc.scalar.copy(sy_row[:, q * Q : (q + 1) * Q], ps[:, :])

                def center_quarter(q):
                    # mu broadcast (1/256 via lhsT), subtract into bf16, then y^2
                    ps = pre.tile([128, Q], F32, tag="pre")
                    for j in range(2):
                        nc.tensor.matmul(
                            ps[:, j * 512 : (j + 1) * 512],
                            lhsT=inv256_row_bf[:, :],
                            rhs=sy_row[:, q * Q + j * 512 : q * Q + (j + 1) * 512],
                            start=True,
                            stop=True,
                        )
                    for k in range(2):
                        nc.vector.tensor_sub(
                            y_bf[:, k, q * Q : (q + 1) * Q],
                            y_bf[:, k, q * Q : (q + 1) * Q],
                            ps[:, :],
                        )
                    nc.scalar.activation(
                        sq[:, :, q * Q : (q + 1) * Q],
                        y_bf[:, :, q * Q : (q + 1) * Q],
                        AF.Square,
                    )

                def center_direct(q):
                    # mu broadcast straight from raw y via ones/256 matmul
                    # (skips the sy row + its Act copy for quarters 2/3)
                    ps = pre.tile([128, Q], F32, tag="pre")
                    for j in range(2):
                        for k in range(2):
                            nc.tensor.matmul(
                                ps[:, j * 512 : (j + 1) * 512],
                                lhsT=oc256_bf[:, :],
                                rhs=y_bf[:, k, q * Q + j * 512 : q * Q + (j + 1) * 512],
                                start=(k == 0),
                                stop=(k == 1),
                            )
                    for k in range(2):
                        nc.vector.tensor_sub(
                            y_bf[:, k, q * Q : (q + 1) * Q],
                            y_bf[:, k, q * Q : (q + 1) * Q],
                            ps[:, :],
                        )
                    nc.scalar.activation(
                        sq[:, :, q * Q : (q + 1) * Q],
                        y_bf[:, :, q * Q : (q + 1) * Q],
                        AF.Square,
                    )

                def qy_quarter(q):
                    ps = pre.tile([1, Q], F32, tag="pre")
                    for k in range(2):
                        for j in range(2):
                            nc.tensor.matmul(
                                ps[:, j * 512 : (j + 1) * 512],
                                lhsT=ones_col_bf[:, :],
                                rhs=sq[:, k, q * Q + j * 512 : q * Q + (j + 1) * 512],
                                start=(k == 0),
                                stop=(k == 1),
                            )
                    nc.scalar.copy(qy_row[:, q * Q : (q + 1) * Q], ps[:, :])

                def invb_quarter(q):
                    ps = pre.tile([128, Q], F32, tag="pre")
                    for j in range(2):
                        nc.tensor.matmul(
                            ps[:, j * 512 : (j + 1) * 512],
                            lhsT=ones_row_bf[:, :],
                            rhs=qy_row[:, q * Q + j * 512 : q * Q + (j + 1) * 512],
                            start=True,
                            stop=True,
                        )
                    nc.scalar.activation(
                        inv_ny_b[:, q * Q : (q + 1) * Q], ps[:, :], AF.Sqrt
                    )
                    with nc.allow_low_precision(reason="inv_ny to bf16 for prescale"):
                        nc.vector.reciprocal(
                            inv_ny_bf[:, q * Q : (q + 1) * Q],
                            inv_ny_b[:, q * Q : (q + 1) * Q],
                        )
                    # pre-scale centered y columns by 1/|Yc| (bf16 2x mode)
                    for k in range(2):
                        nc.vector.tensor_mul(
                            y_bf[:, k, q * Q : (q + 1) * Q],
                            y_bf[:, k, q * Q : (q + 1) * Q],
                            inv_ny_bf[:, q * Q : (q + 1) * Q],
                        )


                def stat16(dst16, src_tile):
                    # dst16[p, t] = sum_c src[c, t*128+p] via N=1 matmuls
                    ps = pre.tile([128, NT], F32, tag="pre")
                    for t in range(NT):
                        for k in range(2):
                            nc.tensor.matmul(
                                ps[:, t : t + 1],
                                lhsT=src_tile[:, k, t * 128 : (t + 1) * 128],
                                rhs=ones_col_bf[:, :],
                                start=(k == 0),
                                stop=(k == 1),
                            )
                    nc.vector.tensor_copy(dst16[:, :], ps[:, :])

                # ---- phase schedule (program order ~ priority) ----------
                y_dma_quarter(0)
                y_dma_quarter(1)
                sy_quarter(0)
                sy_quarter(1)
                center_quarter(0)
                center_quarter(1)
                qy_quarter(0)
                invb_quarter(0)
                qy_quarter(1)
                invb_quarter(1)
                x_quarter(0)
                x_quarter(1)
                xcenter_quarter(0)
                xcenter_quarter(1)
                if os.environ.get("BISECT", "") != "pre":
                    g_quarter(0, 0, d_sbs[0], cmaxs[0])
                stat16(nx2, sq)
                # inv_nx from nx2 (already tile-major)
                nc.scalar.activation(t_b[:, :], nx2[:, :], AF.Sqrt)
                nc.vector.reciprocal(inv_nx[:, :], t_b[:, :])
                nc.vector.tensor_scalar_mul(inv_nx10[:, :], inv_nx[:, :], 10.0)
                y_dma_quarter(2)
                y_dma_quarter(3)
                if os.environ.get("BISECT", "") != "pre":
                    g_quarter(0, 1, d_sbs[0], cmaxs[0])
                center_direct(2)
                center_direct(3)
                qy_quarter(2)
                invb_quarter(2)
                qy_quarter(3)
                invb_quarter(3)
                if os.environ.get("BISECT", "") != "pre":
                    g_quarter(0, 2, d_sbs[0], cmaxs[0])
                    g_quarter(0, 3, d_sbs[0], cmaxs[0])
                    tile_chain_exp(0, d_sbs[0], cmaxs[0])


            # ---- main loop (pools opened above; t=0 quarters already
            # issued inside preprocessing) --------------------------------

            # the preprocessing PSUM pool is closed: its banks back a third
            # and fourth in-flight G buffer for the steady-state loop
            with tc.tile_pool(name="psum_g2", bufs=2, space="PSUM") as psum_g2:
                for t in (range(1, NT) if os.environ.get("BISECT", "") != "pre" else range(0)):
                    d_sb = dpool.tile([128, M], BF16, tag="d_sb")
                    tmp01 = mains.tile([128, Q], BF16, tag="tmp01")
                    tmp23 = mains.tile([128, Q], BF16, tag="tmp23")
                    cmax2 = (tmp01, tmp23)
                    for g in range(4):
                        g_quarter(t, g, d_sb, cmax2,
                                  gpool=psum_g2 if g >= 2 else None)
                    tile_chain_exp(t, d_sb, cmax2)

            # ---- epilogue: v = exp(0.01*r) / sumw -----------------------
            if os.environ.get("BISECT", "") == "pre":
                nc.vector.tensor_copy(v16[:, :], inv_nx[:, :])
            else:
                nc.scalar.activation(maxw16[:, :], r16[:, :], AF.Exp, scale=0.01)
                nc.vector.reciprocal(rs16[:, :], sumw16[:, :])
                nc.vector.tensor_mul(v16[:, :], maxw16[:, :], rs16[:, :])
            nc.sync.dma_start(out=v_d[:, :], in_=v16[:, :])

            mstack.close()

    nc.compile()
    return nc

_NC = None


def _get_nc():
    global _NC
    if _NC is None:
        _NC = build_nc()
    return _NC


def make_in_maps(X, Y):
    """Per-core inputs. Y columns permuted to [own-half | other-half]."""
    in_maps = []
    for c in range(N_CORES):
        b, h = c // 2, c % 2
        xs = np.ascontiguousarray(X[b][:, h * HALF : (h + 1) * HALF])
        ys = np.ascontiguousarray(
            np.concatenate(
                [
                    Y[b][:, h * HALF : (h + 1) * HALF],
                    Y[b][:, (1 - h) * HALF : (2 - h) * HALF],
                ],
                axis=1,
            )
        )
        in_maps.append({"x": xs, "y": ys})
    return in_maps


def finish_host(results):
    """results: list of 8 per-core dicts with 'v' [128, NT]."""
    cx = np.zeros(B, dtype=np.float64)
    for c in range(N_CORES):
        cx[c // 2] += results[c]["v"].astype(np.float64).sum()
    cx /= M
    return np.float32(np.mean(-np.log(cx)))


def run(X_features, Y_features, trace=False, tmpdir=None):
    X = np.asarray(X_features, dtype=np.float32).reshape(B, C, M)
    Y = np.asarray(Y_features, dtype=np.float32).reshape(B, C, M)
    nc = _get_nc()
    res = run_bass_kernel_spmd(
        nc, make_in_maps(X, Y), list(range(N_CORES)), trace=trace, tmpdir=tmpdir
    )
    return finish_host(res.results), res


def kernel(X_features, Y_features):
    loss, _ = run(X_features, Y_features)
    return loss



# revision 78
# speedup vs baseline: 1.0128x; 1.0112x over previous
"""ContextualLoss forward on 8 Trainium2 NeuronCores.

Math (reference):
    mu[m]   = mean_c Y[c, m]                      (PONO over channels of Y)
    Xc = X - mu ; Yc = Y - mu                     (both centered by Y's mean)
    cos[i,j] = <Xc_i, Yc_j> / (|Xc_i| |Yc_j|)
    d = 1 - cos ; dn = d / (min_j d + 1e-3) ; w = exp((1 - dn)/0.1)
    A = w / sum_j w ; CX_b = mean_i max_j A ; loss = mean_b -log CX_b

Device-side tricks:
  * Only Y is centered explicitly. Since Yc has zero channel-mean,
    <Xc_i, Yc_j> == <X_i, Yc_j>, so raw X feeds the matmul.
  * max_j A = exp-at-dmin / sum_j w = exp(0.01/(dmin+1e-3)) / sum_j w
    (w is monotone decreasing in d) -> no second max pass over w.
  * The per-column scale 1/|Yc_j| is pre-applied to the centered Y tile
    (bf16, 2x DVE mode), so PSUM holds d = cos*|Xc_i| directly. The
    PSUM->SBUF drain is split: ScalarE Identity for quarters 0/1, DVE
    copy for quarters 2/3; row maxes via DVE reduce_max on the bf16 d.
    (The fused tensor_tensor_reduce path crashes at runtime on this
    toolchain, so the drain is unfused by design.)
  * Per-row scale 1/|Xc_i| and the softmin exponent fold into the ScalarE
    activation: w = Exp(scale_i * dsc + bias_i), scale_i = s*inv_nx,
    bias_i = 10 - s, s = 10/(dmin+1e-3); accum_out gives sum_j w for free.

Sharding: core c -> sample b = c//2, row-half h = c%2 (2048 rows each).
Each core's Y is column-permuted host-side to [own-half | other-half] so the
identical SPMD program can read the X-half's means from columns [0, 2048).
Row reductions are permutation-invariant, so the permutation is harmless.
"""

import os
import sys
from contextlib import ExitStack

sys.path.insert(0, "/opt/trn_rl_repo")

import numpy as np

import concourse.bass as bass
import concourse.tile as tile
from concourse import bacc
from concourse import mybir
from concourse.bass_utils import run_bass_kernel_spmd

B = 4
C = 256
M = 4096  # 64*64 spatial positions
HALF = M // 2  # rows per core
NT = HALF // 128  # 16 i-tiles per core
N_CORES = 8

F32 = mybir.dt.float32
F32R = mybir.dt.float32r
BF16 = mybir.dt.bfloat16
AF = mybir.ActivationFunctionType
ALU = mybir.AluOpType

NEG_HUGE = -3.0e38


def _r(ap):
    """View a fp32 AP as float32r for full-rate PE matmul."""
    return ap.bitcast(F32R)


def build_nc() -> bass.Bass:
    nc = bacc.Bacc()

    x_d = nc.declare_dram_parameter("x", [C, HALF], F32, isOutput=False)
    y_d = nc.declare_dram_parameter("y", [C, M], F32, isOutput=False)
    v_d = nc.declare_dram_parameter("v", [128, NT], F32, isOutput=True)

    Q = 1024  # preprocessing quarter width

    with tile.TileContext(nc) as tc:
        with (
            tc.tile_pool(name="io", bufs=1) as io,
            tc.tile_pool(name="consts", bufs=1) as consts,
            tc.tile_pool(name="stats", bufs=1) as stats,
        ):
            # ---- inputs -> SBUF: y half 0 first (feeds the sy/center
            # chain), then x, then y half 1 ------------------------------
            x_bf = io.tile([128, 2, HALF], BF16)
            y_bf = io.tile([128, 2, M], BF16)

            # ---- constants ------------------------------------------------
            ones_col = consts.tile([128, 1], F32)
            nc.vector.memset(ones_col, 1.0)
            ones_col_bf = consts.tile([128, 1], BF16)
            nc.vector.memset(ones_col_bf, 1.0)
            ones_row = consts.tile([1, 128], F32)
            nc.vector.memset(ones_row, 1.0)
            ones_row_bf = consts.tile([1, 128], BF16)
            nc.vector.memset(ones_row_bf, 1.0)
            inv256_row_bf = consts.tile([1, 128], BF16)
            nc.vector.memset(inv256_row_bf, 1.0 / 256.0)
            oc256_bf = consts.tile([128, 128], BF16)
            nc.vector.memset(oc256_bf, 1.0 / 256.0)
            ten_col = consts.tile([128, 1], F32)
            nc.vector.memset(ten_col, 10.0)
            one_1x1 = consts.tile([1, 1], F32)
            nc.vector.memset(one_1x1, 1.0)
            one_1x1_bf = consts.tile([1, 1], BF16)
            nc.vector.memset(one_1x1_bf, 1.0)

            inv_ny_b = io.tile([128, M], F32)  # |Yc| then 1/|Yc| broadcast
            inv_ny_bf = io.tile([128, M], BF16)  # bf16 copy for pre-scaling

            nx2 = stats.tile([128, NT], F32)
            inv_nx = stats.tile([128, NT], F32)
            inv_nx10 = stats.tile([128, NT], F32)
            r16 = stats.tile([128, NT], F32)
            sumw16 = stats.tile([128, NT], F32)
            maxw16 = stats.tile([128, NT], F32)
            rs16 = stats.tile([128, NT], F32)
            v16 = stats.tile([128, NT], F32)
            t_b = stats.tile([128, NT], F32)

            # main-loop pools opened alongside preprocessing so tile-0
            # quarters can interleave with the tail of preprocessing
            mstack = ExitStack()
            dpool = mstack.enter_context(tc.tile_pool(name="dpool", bufs=6))
            wpool = mstack.enter_context(tc.tile_pool(name="wpool", bufs=1))
            mains = mstack.enter_context(tc.tile_pool(name="mains", bufs=4))
            psum_g = mstack.enter_context(tc.tile_pool(name="psum_g", bufs=2, space="PSUM"))

            NEARLY = 4
            d_sbs, cmaxs = {}, {}
            for _t in range(NEARLY):
                d_sbs[_t] = dpool.tile([128, M], BF16, tag="d_sb", name=f"d_sb{_t}")
                cmaxs[_t] = (
                    mains.tile([128, Q], BF16, tag="tmp01", name=f"tmp01_{_t}"),
                    mains.tile([128, Q], BF16, tag="tmp23", name=f"tmp23_{_t}"),
                )

            def g_quarter(t, g, d_tile, cmax_tile, gpool=None):
                # y_bf is pre-scaled by 1/|Yc|, so PSUM holds d = cos*|Xc_i|
                # directly: quarters 0/1 drain on Act (Identity), 2/3 on DVE.
                ps = (gpool or psum_g).tile([128, Q], F32, tag="g")
                for k in range(2):
                    for j in range(2):
                        nc.tensor.matmul(
                            ps[:, j * 512 : (j + 1) * 512],
                            lhsT=x_bf[:, k, t * 128 : (t + 1) * 128],
                            rhs=y_bf[:, k, g * Q + j * 512 : g * Q + (j + 1) * 512],
                            start=(k == 0),
                            stop=(k == 1),
                        )
                if g == 0 or (g == 1 and t % 4 != 3):
                    nc.scalar.activation(
                        d_tile[:, g * Q : (g + 1) * Q], ps[:, :], AF.Identity
                    )
                else:
                    nc.vector.tensor_copy(
                        d_tile[:, g * Q : (g + 1) * Q], ps[:, :]
                    )
                # pairwise max tree: bf16 all-SBUF tensor_max runs at 2x
                if g == 1:
                    nc.vector.tensor_max(
                        cmax_tile[0][:, :], d_tile[:, 0:Q], d_tile[:, Q : 2 * Q]
                    )
                elif g == 3:
                    nc.vector.tensor_max(
                        cmax_tile[1][:, :], d_tile[:, 2 * Q : 3 * Q], d_tile[:, 3 * Q : 4 * Q]
                    )

            def tile_chain_exp(t, d_sb, cmax2):
                cmax = mains.tile([128, 1], F32)
                u = mains.tile([128, 1], F32)
                bias_i = mains.tile([128, 1], F32)
                scale_i = mains.tile([128, 1], F32)
                nc.vector.tensor_max(cmax2[0][:, :], cmax2[0][:, :], cmax2[1][:, :])
                nc.vector.tensor_max(
                    cmax2[0][:, 0:512], cmax2[0][:, 0:512], cmax2[0][:, 512:1024]
                )
                nc.vector.tensor_max(
                    cmax2[0][:, 0:256], cmax2[0][:, 0:256], cmax2[0][:, 256:512]
                )
                nc.vector.reduce_max(cmax[:, :], cmax2[0][:, 0:256], axis=mybir.AxisListType.X)
                nc.vector.tensor_mul(cmax[:, :], cmax[:, :], inv_nx[:, t : t + 1])
                nc.vector.tensor_scalar(
                    out=u[:, :],
                    in0=cmax[:, :],
                    scalar1=-1.0,
                    scalar2=1.001,
                    op0=ALU.mult,
                    op1=ALU.add,
                )
                nc.vector.reciprocal(r16[:, t : t + 1], u[:, :])
                nc.vector.tensor_mul(
                    scale_i[:, :], r16[:, t : t + 1], inv_nx10[:, t : t + 1]
                )
                nc.vector.tensor_scalar(
                    out=bias_i[:, :], in0=r16[:, t : t + 1],
                    scalar1=-10.0, scalar2=10.0,
                    op0=ALU.mult, op1=ALU.add,
                )
                w_sb = wpool.tile([128, M], BF16)
                nc.scalar.activation(
                    out=w_sb[:, :],
                    in_=d_sb[:, :],
                    func=AF.Exp,
                    bias=bias_i[:, :],
                    scale=scale_i[:, :],
                    accum_out=sumw16[:, t : t + 1],
                )


            with (
                tc.tile_pool(name="psum_pre", bufs=2, space="PSUM") as pre,
                tc.tile_pool(name="rows", bufs=1) as rows,
                tc.tile_pool(name="scratch", bufs=3) as scratch,
            ):
                # PE warmup to full pstate while DMAs are in flight
                wups = pre.tile([128, 512], F32, tag="pre")
                dumw = consts.tile([128, 1], F32)
                for _wi in range(50):
                    nc.tensor.matmul(
                        wups[:, (_wi % 4) * 128 : (_wi % 4 + 1) * 128],
                        lhsT=ones_row_bf[0:1, :],
                        rhs=ones_row_bf[0:1, :],
                        start=True,
                        stop=True,
                    )
                # Act table preload: Sqrt-set covers Square/Identity/Copy
                nc.scalar.activation(dumw[:, :], ten_col[:, :], AF.Sqrt)
                sy_row = rows.tile([1, M], BF16, tag="rowM")
                qy_row = rows.tile([1, M], BF16, tag="rowM2")
                sq = rows.tile([128, 2, M], BF16)  # squares staging (bf16: only feeds aggregate norms)

                ystage = {}

                y_v = y_d.rearrange("(k p) m -> p k m", p=128)
                x_v = x_d.rearrange("(k p) m -> p k m", p=128)

                def y_dma_quarter(q):
                    st = scratch.tile([128, 2, Q], F32, tag="stage")
                    nc.sync.dma_start(
                        out=st[:, :, :], in_=y_v[:, :, q * Q : (q + 1) * Q]
                    )
                    # raw y -> bf16 (centered in place later)
                    nc.vector.tensor_copy(
                        y_bf[:, :, q * Q : (q + 1) * Q], st[:, :, :]
                    )
                    ystage[q] = st

                def x_quarter(q):
                    st = scratch.tile([128, 2, Q], F32, tag="stage")
                    nc.sync.dma_start(
                        out=st[:, :, :], in_=x_v[:, :, q * Q : (q + 1) * Q]
                    )
                    # Pool is idle in the preamble; keep Act free for the
                    # y-quarter chains
                    nc.gpsimd.tensor_copy(
                        x_bf[:, :, q * Q : (q + 1) * Q], st[:, :, :]
                    )

                def xcenter_quarter(q):
                    # x_bf -= mu (in place); then xc^2 -> sq (overwrites y^2
                    # region after qy MMs have consumed it)
                    ps = pre.tile([128, Q], F32, tag="pre")
                    for j in range(2):
                        nc.tensor.matmul(
                            ps[:, j * 512 : (j + 1) * 512],
                            lhsT=inv256_row_bf[:, :],
                            rhs=sy_row[:, q * Q + j * 512 : q * Q + (j + 1) * 512],
                            start=True,
                            stop=True,
                        )
                    for k in range(2):
                        nc.vector.tensor_sub(
                            x_bf[:, k, q * Q : (q + 1) * Q],
                            x_bf[:, k, q * Q : (q + 1) * Q],
                            ps[:, :],
                        )
                    nc.scalar.activation(
                        sq[:, :, q * Q : (q + 1) * Q],
                        x_bf[:, :, q * Q : (q + 1) * Q],
                        AF.Square,
                    )

                def sy_quarter(q):
                    ps = pre.tile([1, Q], F32, tag="pre")
                    for k in range(2):
                        for j in range(2):
                            nc.tensor.matmul(
                                ps[:, j * 512 : (j + 1) * 512],
                                lhsT=ones_col_bf[:, :],
                                rhs=y_bf[:, k, q * Q + j * 512 : q * Q + (j + 1) * 512],
                                start=(k == 0),
                                stop=(k == 1),
                            )
                    nc.scalar.copy(sy_row[:, q * Q : (q + 1) * Q], ps[:, :])

                def center_quarter(q):
                    # mu broadcast (1/256 via lhsT), subtract into bf16, then y^2
                    ps = pre.tile([128, Q], F32, tag="pre")
                    for j in range(2):
                        nc.tensor.matmul(
                            ps[:, j * 512 : (j + 1) * 512],
                            lhsT=inv256_row_bf[:, :],
                            rhs=sy_row[:, q * Q + j * 512 : q * Q + (j + 1) * 512],
                            start=True,
                            stop=True,
                        )
                    for k in range(2):
                        nc.vector.tensor_sub(
                            y_bf[:, k, q * Q : (q + 1) * Q],
                            y_bf[:, k, q * Q : (q + 1) * Q],
                            ps[:, :],
                        )
                    nc.scalar.activation(
                        sq[:, :, q * Q : (q + 1) * Q],
                        y_bf[:, :, q * Q : (q + 1) * Q],
                        AF.Square,
                    )

                def center_direct(q):
                    # mu broadcast straight from raw y via ones/256 matmul
                    # (skips the sy row + its Act copy for quarters 2/3)
                    ps = pre.tile([128, Q], F32, tag="pre")
                    for j in range(2):
                        for k in range(2):
                            nc.tensor.matmul(
                                ps[:, j * 512 : (j + 1) * 512],
                                lhsT=oc256_bf[:, :],
                                rhs=y_bf[:, k, q * Q + j * 512 : q * Q + (j + 1) * 512],
                                start=(k == 0),
                                stop=(k == 1),
                            )
                    for k in range(2):
                        nc.vector.tensor_sub(
                            y_bf[:, k, q * Q : (q + 1) * Q],
                            y_bf[:, k, q * Q : (q + 1) * Q],
                            ps[:, :],
                        )
                    nc.scalar.activation(
                        sq[:, :, q * Q : (q + 1) * Q],
                        y_bf[:, :, q * Q : (q + 1) * Q],
                        AF.Square,
                    )

                def qy_quarter(q):
                    ps = pre.tile([1, Q], F32, tag="pre")
                    for k in range(2):
                        for j in range(2):
                            nc.tensor.matmul(
                                ps[:, j * 512 : (j + 1) * 512],
                                lhsT=ones_col_bf[:, :],
                                rhs=sq[:, k, q * Q + j * 512 : q * Q + (j + 1) * 512],
                                start=(k == 0),
                                stop=(k == 1),
                            )
                    nc.scalar.copy(qy_row[:, q * Q : (q + 1) * Q], ps[:, :])

                def invb_quarter(q):
                    ps = pre.tile([128, Q], F32, tag="pre")
                    for j in range(2):
                        nc.tensor.matmul(
                            ps[:, j * 512 : (j + 1) * 512],
                            lhsT=ones_row_bf[:, :],
                            rhs=qy_row[:, q * Q + j * 512 : q * Q + (j + 1) * 512],
                            start=True,
                            stop=True,
                        )
                    nc.scalar.activation(
                        inv_ny_b[:, q * Q : (q + 1) * Q], ps[:, :], AF.Sqrt
                    )
                    with nc.allow_low_precision(reason="inv_ny to bf16 for prescale"):
                        nc.vector.reciprocal(
                            inv_ny_bf[:, q * Q : (q + 1) * Q],
                            inv_ny_b[:, q * Q : (q + 1) * Q],
                        )
                    # pre-scale centered y columns by 1/|Yc| (bf16 2x mode)
                    for k in range(2):
                        nc.vector.tensor_mul(
                            y_bf[:, k, q * Q : (q + 1) * Q],
                            y_bf[:, k, q * Q : (q + 1) * Q],
                            inv_ny_bf[:, q * Q : (q + 1) * Q],
                        )


                def stat16(dst16, src_tile):
                    # dst16[p, t] = sum_c src[c, t*128+p] via N=1 matmuls
                    ps = pre.tile([128, NT], F32, tag="pre")
                    for t in range(NT):
                        for k in range(2):
                            nc.tensor.matmul(
                                ps[:, t : t + 1],
                                lhsT=src_tile[:, k, t * 128 : (t + 1) * 128],
                                rhs=ones_col_bf[:, :],
                                start=(k == 0),
                                stop=(k == 1),
                            )
                    nc.vector.tensor_copy(dst16[:, :], ps[:, :])

                # ---- phase schedule (program order ~ priority) ----------
                y_dma_quarter(0)
                y_dma_quarter(1)
                sy_quarter(0)
                sy_quarter(1)
                center_quarter(0)
                center_quarter(1)
                qy_quarter(0)
                invb_quarter(0)
                qy_quarter(1)
                invb_quarter(1)
                x_quarter(0)
                x_quarter(1)
                xcenter_quarter(0)
                xcenter_quarter(1)
                if os.environ.get("BISECT", "") != "pre":
                    g_quarter(0, 0, d_sbs[0], cmaxs[0])
                stat16(nx2, sq)
                # inv_nx from nx2 (already tile-major)
                nc.scalar.activation(t_b[:, :], nx2[:, :], AF.Sqrt)
                nc.vector.reciprocal(inv_nx[:, :], t_b[:, :])
                nc.vector.tensor_scalar_mul(inv_nx10[:, :], inv_nx[:, :], 10.0)
                y_dma_quarter(2)
                y_dma_quarter(3)
                if os.environ.get("BISECT", "") != "pre":
                    g_quarter(0, 1, d_sbs[0], cmaxs[0])
                center_direct(2)
                center_direct(3)
                qy_quarter(2)
                invb_quarter(2)
                qy_quarter(3)
                invb_quarter(3)
                # preload the Exp table before the first real exp; the data
                # dependency on the q3 norms pins it after the LAST Sqrt
                nc.scalar.activation(dumw[:, :], inv_ny_b[:, M - 1 : M], AF.Exp)
                if os.environ.get("BISECT", "") != "pre":
                    g_quarter(0, 2, d_sbs[0], cmaxs[0])
                    g_quarter(0, 3, d_sbs[0], cmaxs[0])
                    tile_chain_exp(0, d_sbs[0], cmaxs[0])


            # ---- main loop (pools opened above; t=0 quarters already
            # issued inside preprocessing) --------------------------------

            # the preprocessing PSUM pool is closed: its banks back a third
            # and fourth in-flight G buffer for the steady-state loop
            with tc.tile_pool(name="psum_g2", bufs=2, space="PSUM") as psum_g2:
                for t in (range(1, NT) if os.environ.get("BISECT", "") != "pre" else range(0)):
                    d_sb = dpool.tile([128, M], BF16, tag="d_sb")
                    tmp01 = mains.tile([128, Q], BF16, tag="tmp01")
                    tmp23 = mains.tile([128, Q], BF16, tag="tmp23")
                    cmax2 = (tmp01, tmp23)
                    for g in range(4):
                        g_quarter(t, g, d_sb, cmax2,
                                  gpool=psum_g2 if g >= 2 else None)
                    tile_chain_exp(t, d_sb, cmax2)

            # ---- epilogue: v = exp(0.01*r) / sumw -----------------------
            if os.environ.get("BISECT", "") == "pre":
                nc.vector.tensor_copy(v16[:, :], inv_nx[:, :])
            else:
                nc.scalar.activation(maxw16[:, :], r16[:, :], AF.Exp, scale=0.01)
                nc.vector.reciprocal(rs16[:, :], sumw16[:, :])
                nc.vector.tensor_mul(v16[:, :], maxw16[:, :], rs16[:, :])
            nc.sync.dma_start(out=v_d[:, :], in_=v16[:, :])

            mstack.close()

    nc.compile()
    return nc

_NC = None


def _get_nc():
    global _NC
    if _NC is None:
        _NC = build_nc()
    return _NC


def make_in_maps(X, Y):
    """Per-core inputs. Y columns permuted to [own-half | other-half]."""
    in_maps = []
    for c in range(N_CORES):
        b, h = c // 2, c % 2
        xs = np.ascontiguousarray(X[b][:, h * HALF : (h + 1) * HALF])
        ys = np.ascontiguousarray(
            np.concatenate(
                [
                    Y[b][:, h * HALF : (h + 1) * HALF],
                    Y[b][:, (1 - h) * HALF : (2 - h) * HALF],
                ],
                axis=1,
            )
        )
        in_maps.append({"x": xs, "y": ys})
    return in_maps


def finish_host(results):
    """results: list of 8 per-core dicts with 'v' [128, NT]."""
    cx = np.zeros(B, dtype=np.float64)
    for c in range(N_CORES):
        cx[c // 2] += results[c]["v"].astype(np.float64).sum()
    cx /= M
    return np.float32(np.mean(-np.log(cx)))


def run(X_features, Y_features, trace=False, tmpdir=None):
    X = np.asarray(X_features, dtype=np.float32).reshape(B, C, M)
    Y = np.asarray(Y_features, dtype=np.float32).reshape(B, C, M)
    nc = _get_nc()
    res = run_bass_kernel_spmd(
        nc, make_in_maps(X, Y), list(range(N_CORES)), trace=trace, tmpdir=tmpdir
    )
    return finish_host(res.results), res


def kernel(X_features, Y_features):
    loss, _ = run(X_features, Y_features)
    return loss



# revision 82
# speedup vs baseline: 1.0215x; 1.0086x over previous
"""ContextualLoss forward on 8 Trainium2 NeuronCores.

Math (reference):
    mu[m]   = mean_c Y[c, m]                      (PONO over channels of Y)
    Xc = X - mu ; Yc = Y - mu                     (both centered by Y's mean)
    cos[i,j] = <Xc_i, Yc_j> / (|Xc_i| |Yc_j|)
    d = 1 - cos ; dn = d / (min_j d + 1e-3) ; w = exp((1 - dn)/0.1)
    A = w / sum_j w ; CX_b = mean_i max_j A ; loss = mean_b -log CX_b

Device-side tricks:
  * Only Y is centered explicitly. Since Yc has zero channel-mean,
    <Xc_i, Yc_j> == <X_i, Yc_j>, so raw X feeds the matmul.
  * max_j A = exp-at-dmin / sum_j w = exp(0.01/(dmin+1e-3)) / sum_j w
    (w is monotone decreasing in d) -> no second max pass over w.
  * The per-column scale 1/|Yc_j| is pre-applied to the centered Y tile
    (bf16, 2x DVE mode), so PSUM holds d = cos*|Xc_i| directly. The
    PSUM->SBUF drain is split: ScalarE Identity for quarters 0/1, DVE
    copy for quarters 2/3; row maxes via DVE reduce_max on the bf16 d.
    (The fused tensor_tensor_reduce path crashes at runtime on this
    toolchain, so the drain is unfused by design.)
  * Per-row scale 1/|Xc_i| and the softmin exponent fold into the ScalarE
    activation: w = Exp(scale_i * dsc + bias_i), scale_i = s*inv_nx,
    bias_i = 10 - s, s = 10/(dmin+1e-3); accum_out gives sum_j w for free.

Sharding: core c -> sample b = c//2, row-half h = c%2 (2048 rows each).
Each core's Y is column-permuted host-side to [own-half | other-half] so the
identical SPMD program can read the X-half's means from columns [0, 2048).
Row reductions are permutation-invariant, so the permutation is harmless.
"""

import os
import sys
from contextlib import ExitStack

sys.path.insert(0, "/opt/trn_rl_repo")

import numpy as np

import concourse.bass as bass
import concourse.tile as tile
from concourse import bacc
from concourse import mybir
from concourse.bass_utils import run_bass_kernel_spmd

B = 4
C = 256
M = 4096  # 64*64 spatial positions
HALF = M // 2  # rows per core
NT = HALF // 128  # 16 i-tiles per core
N_CORES = 8

F32 = mybir.dt.float32
F32R = mybir.dt.float32r
BF16 = mybir.dt.bfloat16
AF = mybir.ActivationFunctionType
ALU = mybir.AluOpType

NEG_HUGE = -3.0e38


def _r(ap):
    """View a fp32 AP as float32r for full-rate PE matmul."""
    return ap.bitcast(F32R)


def build_nc() -> bass.Bass:
    nc = bacc.Bacc()

    x_d = nc.declare_dram_parameter("x", [C, HALF], F32, isOutput=False)
    y_d = nc.declare_dram_parameter("y", [C, M], F32, isOutput=False)
    v_d = nc.declare_dram_parameter("v", [128, NT], F32, isOutput=True)

    Q = 1024  # preprocessing quarter width

    with tile.TileContext(nc) as tc:
        with (
            tc.tile_pool(name="io", bufs=1) as io,
            tc.tile_pool(name="consts", bufs=1) as consts,
            tc.tile_pool(name="stats", bufs=1) as stats,
        ):
            # ---- inputs -> SBUF: y half 0 first (feeds the sy/center
            # chain), then x, then y half 1 ------------------------------
            x_bf = io.tile([128, 2, HALF], BF16)
            y_bf = io.tile([128, 2, M], BF16)

            # ---- constants ------------------------------------------------
            ones_col = consts.tile([128, 1], F32)
            nc.vector.memset(ones_col, 1.0)
            ones_col_bf = consts.tile([128, 1], BF16)
            nc.vector.memset(ones_col_bf, 1.0)
            ones_row = consts.tile([1, 128], F32)
            nc.vector.memset(ones_row, 1.0)
            ones_row_bf = consts.tile([1, 128], BF16)
            nc.vector.memset(ones_row_bf, 1.0)
            inv256_row_bf = consts.tile([1, 128], BF16)
            nc.vector.memset(inv256_row_bf, 1.0 / 256.0)
            oc256_bf = consts.tile([128, 128], BF16)
            nc.vector.memset(oc256_bf, 1.0 / 256.0)
            ten_col = consts.tile([128, 1], F32)
            nc.vector.memset(ten_col, 10.0)
            one_1x1 = consts.tile([1, 1], F32)
            nc.vector.memset(one_1x1, 1.0)
            one_1x1_bf = consts.tile([1, 1], BF16)
            nc.vector.memset(one_1x1_bf, 1.0)

            inv_ny_b = io.tile([128, M], F32)  # |Yc| then 1/|Yc| broadcast
            inv_ny_bf = io.tile([128, M], BF16)  # bf16 copy for pre-scaling

            nx2 = stats.tile([128, NT], F32)
            inv_nx = stats.tile([128, NT], F32)
            inv_nx10 = stats.tile([128, NT], F32)
            r16 = stats.tile([128, NT], F32)
            sumw16 = stats.tile([128, NT], F32)
            maxw16 = stats.tile([128, NT], F32)
            rs16 = stats.tile([128, NT], F32)
            v16 = stats.tile([128, NT], F32)
            t_b = stats.tile([128, NT], F32)

            # main-loop pools opened alongside preprocessing so tile-0
            # quarters can interleave with the tail of preprocessing
            mstack = ExitStack()
            dpool = mstack.enter_context(tc.tile_pool(name="dpool", bufs=6))
            wpool = mstack.enter_context(tc.tile_pool(name="wpool", bufs=1))
            mains = mstack.enter_context(tc.tile_pool(name="mains", bufs=4))
            psum_g = mstack.enter_context(tc.tile_pool(name="psum_g", bufs=2, space="PSUM"))

            NEARLY = 4
            d_sbs, cmaxs = {}, {}
            for _t in range(NEARLY):
                d_sbs[_t] = dpool.tile([128, M], BF16, tag="d_sb", name=f"d_sb{_t}")
                cmaxs[_t] = (
                    mains.tile([128, Q], BF16, tag="tmp01", name=f"tmp01_{_t}"),
                    mains.tile([128, Q], BF16, tag="tmp23", name=f"tmp23_{_t}"),
                )

            def g_quarter(t, g, d_tile, cmax_tile, gpool=None):
                # y_bf is pre-scaled by 1/|Yc|, so PSUM holds d = cos*|Xc_i|
                # directly: quarters 0/1 drain on Act (Identity), 2/3 on DVE.
                ps = (gpool or psum_g).tile([128, Q], F32, tag="g")
                for k in range(2):
                    for j in range(2):
                        nc.tensor.matmul(
                            ps[:, j * 512 : (j + 1) * 512],
                            lhsT=x_bf[:, k, t * 128 : (t + 1) * 128],
                            rhs=y_bf[:, k, g * Q + j * 512 : g * Q + (j + 1) * 512],
                            start=(k == 0),
                            stop=(k == 1),
                        )
                if g == 0 or (g == 1 and t % 4 != 3):
                    nc.scalar.activation(
                        d_tile[:, g * Q : (g + 1) * Q], ps[:, :], AF.Identity
                    )
                else:
                    nc.vector.tensor_copy(
                        d_tile[:, g * Q : (g + 1) * Q], ps[:, :]
                    )
                # pairwise max tree: bf16 all-SBUF tensor_max runs at 2x
                if g == 1:
                    nc.vector.tensor_max(
                        cmax_tile[0][:, :], d_tile[:, 0:Q], d_tile[:, Q : 2 * Q]
                    )
                elif g == 3:
                    nc.vector.tensor_max(
                        cmax_tile[1][:, :], d_tile[:, 2 * Q : 3 * Q], d_tile[:, 3 * Q : 4 * Q]
                    )

            def tile_chain_exp(t, d_sb, cmax2):
                cmax = mains.tile([128, 1], F32)
                u = mains.tile([128, 1], F32)
                bias_i = mains.tile([128, 1], F32)
                scale_i = mains.tile([128, 1], F32)
                nc.vector.tensor_max(cmax2[0][:, :], cmax2[0][:, :], cmax2[1][:, :])
                nc.vector.tensor_max(
                    cmax2[0][:, 0:512], cmax2[0][:, 0:512], cmax2[0][:, 512:1024]
                )
                nc.vector.tensor_max(
                    cmax2[0][:, 0:256], cmax2[0][:, 0:256], cmax2[0][:, 256:512]
                )
                nc.vector.reduce_max(cmax[:, :], cmax2[0][:, 0:256], axis=mybir.AxisListType.X)
                nc.vector.tensor_mul(cmax[:, :], cmax[:, :], inv_nx[:, t : t + 1])
                nc.vector.tensor_scalar(
                    out=u[:, :],
                    in0=cmax[:, :],
                    scalar1=-1.0,
                    scalar2=1.001,
                    op0=ALU.mult,
                    op1=ALU.add,
                )
                nc.vector.reciprocal(r16[:, t : t + 1], u[:, :])
                nc.vector.tensor_mul(
                    scale_i[:, :], r16[:, t : t + 1], inv_nx10[:, t : t + 1]
                )
                nc.vector.tensor_scalar(
                    out=bias_i[:, :], in0=r16[:, t : t + 1],
                    scalar1=-10.0, scalar2=10.0,
                    op0=ALU.mult, op1=ALU.add,
                )
                w_sb = wpool.tile([128, M], BF16)
                nc.scalar.activation(
                    out=w_sb[:, :],
                    in_=d_sb[:, :],
                    func=AF.Exp,
                    bias=bias_i[:, :],
                    scale=scale_i[:, :],
                    accum_out=sumw16[:, t : t + 1],
                )


            with (
                tc.tile_pool(name="psum_pre", bufs=2, space="PSUM") as pre,
                tc.tile_pool(name="rows", bufs=1) as rows,
                tc.tile_pool(name="scratch", bufs=3) as scratch,
            ):
                # PE warmup to full pstate while DMAs are in flight
                wups = pre.tile([128, 512], F32, tag="pre")
                dumw = consts.tile([128, 1], F32)
                for _wi in range(50):
                    nc.tensor.matmul(
                        wups[:, (_wi % 4) * 128 : (_wi % 4 + 1) * 128],
                        lhsT=ones_row_bf[0:1, :],
                        rhs=ones_row_bf[0:1, :],
                        start=True,
                        stop=True,
                    )
                # Act table preload: Sqrt-set covers Square/Identity/Copy
                nc.scalar.activation(dumw[:, :], ten_col[:, :], AF.Sqrt)
                sy_row = rows.tile([1, M], BF16, tag="rowM")
                qy_row = rows.tile([1, M], BF16, tag="rowM2")
                sq = rows.tile([128, 2, M], BF16)  # squares staging (bf16: only feeds aggregate norms)

                ystage = {}

                y_v = y_d.rearrange("(k p) m -> p k m", p=128)
                x_v = x_d.rearrange("(k p) m -> p k m", p=128)

                def y_dma_quarter(q):
                    # half-width DMAs + copies: downstream j-block matmuls
                    # start on the first half ~1.4us earlier
                    st = scratch.tile([128, 2, Q], F32, tag="stage")
                    for h in range(2):
                        nc.sync.dma_start(
                            out=st[:, :, h * 512 : (h + 1) * 512],
                            in_=y_v[:, :, q * Q + h * 512 : q * Q + (h + 1) * 512],
                        )
                        nc.vector.tensor_copy(
                            y_bf[:, :, q * Q + h * 512 : q * Q + (h + 1) * 512],
                            st[:, :, h * 512 : (h + 1) * 512],
                        )
                    ystage[q] = st

                def x_quarter(q):
                    st = scratch.tile([128, 2, Q], F32, tag="stage")
                    nc.sync.dma_start(
                        out=st[:, :, :], in_=x_v[:, :, q * Q : (q + 1) * Q]
                    )
                    # Pool is idle in the preamble; keep Act free for the
                    # y-quarter chains
                    nc.gpsimd.tensor_copy(
                        x_bf[:, :, q * Q : (q + 1) * Q], st[:, :, :]
                    )

                def xcenter_quarter(q):
                    # x_bf -= mu (in place); then xc^2 -> sq (overwrites y^2
                    # region after qy MMs have consumed it)
                    ps = pre.tile([128, Q], F32, tag="pre")
                    for j in range(2):
                        nc.tensor.matmul(
                            ps[:, j * 512 : (j + 1) * 512],
                            lhsT=inv256_row_bf[:, :],
                            rhs=sy_row[:, q * Q + j * 512 : q * Q + (j + 1) * 512],
                            start=True,
                            stop=True,
                        )
                    for k in range(2):
                        nc.vector.tensor_sub(
                            x_bf[:, k, q * Q : (q + 1) * Q],
                            x_bf[:, k, q * Q : (q + 1) * Q],
                            ps[:, :],
                        )
                    nc.scalar.activation(
                        sq[:, :, q * Q : (q + 1) * Q],
                        x_bf[:, :, q * Q : (q + 1) * Q],
                        AF.Square,
                    )

                def sy_quarter(q):
                    ps = pre.tile([1, Q], F32, tag="pre")
                    for k in range(2):
                        for j in range(2):
                            nc.tensor.matmul(
                                ps[:, j * 512 : (j + 1) * 512],
                                lhsT=ones_col_bf[:, :],
                                rhs=y_bf[:, k, q * Q + j * 512 : q * Q + (j + 1) * 512],
                                start=(k == 0),
                                stop=(k == 1),
                            )
                    nc.scalar.copy(sy_row[:, q * Q : (q + 1) * Q], ps[:, :])

                def center_quarter(q):
                    # mu broadcast (1/256 via lhsT), subtract into bf16, then y^2
                    ps = pre.tile([128, Q], F32, tag="pre")
                    for j in range(2):
                        nc.tensor.matmul(
                            ps[:, j * 512 : (j + 1) * 512],
                            lhsT=inv256_row_bf[:, :],
                            rhs=sy_row[:, q * Q + j * 512 : q * Q + (j + 1) * 512],
                            start=True,
                            stop=True,
                        )
                    for k in range(2):
                        nc.vector.tensor_sub(
                            y_bf[:, k, q * Q : (q + 1) * Q],
                            y_bf[:, k, q * Q : (q + 1) * Q],
                            ps[:, :],
                        )
                    nc.scalar.activation(
                        sq[:, :, q * Q : (q + 1) * Q],
                        y_bf[:, :, q * Q : (q + 1) * Q],
                        AF.Square,
                    )

                def center_direct(q):
                    # mu broadcast straight from raw y via ones/256 matmul
                    # (skips the sy row + its Act copy for quarters 2/3)
                    ps = pre.tile([128, Q], F32, tag="pre")
                    for j in range(2):
                        for k in range(2):
                            nc.tensor.matmul(
                                ps[:, j * 512 : (j + 1) * 512],
                                lhsT=oc256_bf[:, :],
                                rhs=y_bf[:, k, q * Q + j * 512 : q * Q + (j + 1) * 512],
                                start=(k == 0),
                                stop=(k == 1),
                            )
                    for k in range(2):
                        nc.vector.tensor_sub(
                            y_bf[:, k, q * Q : (q + 1) * Q],
                            y_bf[:, k, q * Q : (q + 1) * Q],
                            ps[:, :],
                        )
                    nc.scalar.activation(
                        sq[:, :, q * Q : (q + 1) * Q],
                        y_bf[:, :, q * Q : (q + 1) * Q],
                        AF.Square,
                    )

                def qy_quarter(q):
                    ps = pre.tile([1, Q], F32, tag="pre")
                    for k in range(2):
                        for j in range(2):
                            nc.tensor.matmul(
                                ps[:, j * 512 : (j + 1) * 512],
                                lhsT=ones_col_bf[:, :],
                                rhs=sq[:, k, q * Q + j * 512 : q * Q + (j + 1) * 512],
                                start=(k == 0),
                                stop=(k == 1),
                            )
                    nc.scalar.copy(qy_row[:, q * Q : (q + 1) * Q], ps[:, :])

                def invb_quarter(q):
                    ps = pre.tile([128, Q], F32, tag="pre")
                    for j in range(2):
                        nc.tensor.matmul(
                            ps[:, j * 512 : (j + 1) * 512],
                            lhsT=ones_row_bf[:, :],
                            rhs=qy_row[:, q * Q + j * 512 : q * Q + (j + 1) * 512],
                            start=True,
                            stop=True,
                        )
                    nc.scalar.activation(
                        inv_ny_b[:, q * Q : (q + 1) * Q], ps[:, :], AF.Sqrt
                    )
                    with nc.allow_low_precision(reason="inv_ny to bf16 for prescale"):
                        nc.vector.reciprocal(
                            inv_ny_bf[:, q * Q : (q + 1) * Q],
                            inv_ny_b[:, q * Q : (q + 1) * Q],
                        )
                    # pre-scale centered y columns by 1/|Yc| (bf16 2x mode)
                    for k in range(2):
                        nc.vector.tensor_mul(
                            y_bf[:, k, q * Q : (q + 1) * Q],
                            y_bf[:, k, q * Q : (q + 1) * Q],
                            inv_ny_bf[:, q * Q : (q + 1) * Q],
                        )


                def stat16(dst16, src_tile):
                    # dst16[p, t] = sum_c src[c, t*128+p] via N=1 matmuls
                    ps = pre.tile([128, NT], F32, tag="pre")
                    for t in range(NT):
                        for k in range(2):
                            nc.tensor.matmul(
                                ps[:, t : t + 1],
                                lhsT=src_tile[:, k, t * 128 : (t + 1) * 128],
                                rhs=ones_col_bf[:, :],
                                start=(k == 0),
                                stop=(k == 1),
                            )
                    nc.vector.tensor_copy(dst16[:, :], ps[:, :])

                # ---- phase schedule (program order ~ priority) ----------
                y_dma_quarter(0)
                y_dma_quarter(1)
                sy_quarter(0)
                sy_quarter(1)
                center_quarter(0)
                center_quarter(1)
                qy_quarter(0)
                invb_quarter(0)
                qy_quarter(1)
                invb_quarter(1)
                x_quarter(0)
                x_quarter(1)
                xcenter_quarter(0)
                xcenter_quarter(1)
                if os.environ.get("BISECT", "") != "pre":
                    g_quarter(0, 0, d_sbs[0], cmaxs[0])
                stat16(nx2, sq)
                # inv_nx from nx2 (already tile-major)
                nc.scalar.activation(t_b[:, :], nx2[:, :], AF.Sqrt)
                nc.vector.reciprocal(inv_nx[:, :], t_b[:, :])
                nc.vector.tensor_scalar_mul(inv_nx10[:, :], inv_nx[:, :], 10.0)
                y_dma_quarter(2)
                y_dma_quarter(3)
                if os.environ.get("BISECT", "") != "pre":
                    g_quarter(0, 1, d_sbs[0], cmaxs[0])
                center_direct(2)
                center_direct(3)
                qy_quarter(2)
                invb_quarter(2)
                qy_quarter(3)
                invb_quarter(3)
                # preload the Exp table before the first real exp; the data
                # dependency on the q3 norms pins it after the LAST Sqrt
                nc.scalar.activation(dumw[:, :], inv_ny_b[:, M - 1 : M], AF.Exp)
                if os.environ.get("BISECT", "") != "pre":
                    g_quarter(0, 2, d_sbs[0], cmaxs[0])
                    g_quarter(0, 3, d_sbs[0], cmaxs[0])
                    tile_chain_exp(0, d_sbs[0], cmaxs[0])


            # ---- main loop (pools opened above; t=0 quarters already
            # issued inside preprocessing) --------------------------------

            # the preprocessing PSUM pool is closed: its banks back a third
            # and fourth in-flight G buffer for the steady-state loop
            with tc.tile_pool(name="psum_g2", bufs=2, space="PSUM") as psum_g2:
                for t in (range(1, NT) if os.environ.get("BISECT", "") != "pre" else range(0)):
                    d_sb = dpool.tile([128, M], BF16, tag="d_sb")
                    tmp01 = mains.tile([128, Q], BF16, tag="tmp01")
                    tmp23 = mains.tile([128, Q], BF16, tag="tmp23")
                    cmax2 = (tmp01, tmp23)
                    for g in range(4):
                        g_quarter(t, g, d_sb, cmax2,
                                  gpool=psum_g2 if g >= 2 else None)
                    tile_chain_exp(t, d_sb, cmax2)

            # ---- epilogue: v = exp(0.01*r) / sumw -----------------------
            if os.environ.get("BISECT", "") == "pre":
                nc.vector.tensor_copy(v16[:, :], inv_nx[:, :])
            else:
                nc.scalar.activation(maxw16[:, :], r16[:, :], AF.Exp, scale=0.01)
                nc.vector.reciprocal(rs16[:, :], sumw16[:, :])
                nc.vector.tensor_mul(v16[:, :], maxw16[:, :], rs16[:, :])
            nc.sync.dma_start(out=v_d[:, :], in_=v16[:, :])

            mstack.close()

    nc.compile()
    return nc

_NC = None


def _get_nc():
    global _NC
    if _NC is None:
        _NC = build_nc()
    return _NC


def make_in_maps(X, Y):
    """Per-core inputs. Y columns permuted to [own-half | other-half]."""
    in_maps = []
    for c in range(N_CORES):
        b, h = c // 2, c % 2
        xs = np.ascontiguousarray(X[b][:, h * HALF : (h + 1) * HALF])
        ys = np.ascontiguousarray(
            np.concatenate(
                [
                    Y[b][:, h * HALF : (h + 1) * HALF],
                    Y[b][:, (1 - h) * HALF : (2 - h) * HALF],
                ],
                axis=1,
            )
        )
        in_maps.append({"x": xs, "y": ys})
    return in_maps


def finish_host(results):
    """results: list of 8 per-core dicts with 'v' [128, NT]."""
    cx = np.zeros(B, dtype=np.float64)
    for c in range(N_CORES):
        cx[c // 2] += results[c]["v"].astype(np.float64).sum()
    cx /= M
    return np.float32(np.mean(-np.log(cx)))


def run(X_features, Y_features, trace=False, tmpdir=None):
    X = np.asarray(X_features, dtype=np.float32).reshape(B, C, M)
    Y = np.asarray(Y_features, dtype=np.float32).reshape(B, C, M)
    nc = _get_nc()
    res = run_bass_kernel_spmd(
        nc, make_in_maps(X, Y), list(range(N_CORES)), trace=trace, tmpdir=tmpdir
    )
    return finish_host(res.results), res


def kernel(X_features, Y_features):
    loss, _ = run(X_features, Y_features)
    return loss

